# revision 18
# baseline (speedup 1.0000x reference)
"""Fused sp2norm-MHA kernel for Trainium2, 8 NeuronCores.

Model (per reference):
    qkv = x @ W_qkv.T ; split heads (H=16, hs=64)
    s = (q @ k.T) / sqrt(hs);  w = softplus(s) causal-masked
    out_h = (w @ v) / ||w||_row ;  out = concat(out_h) @ W_proj.T + b_proj

Sharding: core c = (b, g) with b = c // 4 (batch), g = c % 4 (head group of 4).
Each core computes its batch's QKV for its 4 heads, the attention, and a
partial projection over its 256 feature channels. The host sums the 4 partial
projections per batch and adds the bias (the unshard step).

On-device layout (per core):
    xT  [1024, 2048]  bf16  = x[b].T                  (c on partitions)
    Sᵀ attention: scores computed transposed [j, i] (keys on partitions) so
    softplus output w feeds (a) out.T = V.T-style matmul lhsT=V[j,d], and
    (b) norm² = ones.T @ w² — both contract over j on partitions.
    softplus = Exp (scale=1/8, PSUM src) then Ln (bias=1.0), fp16 intermediate.
    Causality: block-ragged i-range per j-block; the diagonal 128-col
    sub-block is masked on the fp16 exp intermediate (Ln(e*m + 1) = 0).

v3 structure: the scalar engine (softplus Exp+Ln, ~1 elem/cycle/lane) is the
roofline at ~150us; everything is arranged so it starts early and never
stalls:
  - PSUM roles are disjoint: the score pool (4 banks) is never used by the
    QKV phase, so the first attention chunk's scores/Exp run ~10us into the
    kernel instead of waiting for the whole QKV phase to drain (the baseline
    lost ~45us to exactly this WAR-on-bank-reuse serialization).
  - Emission interleaves QKV "feeds" with attention chunks in dependency
    order (ring allocation order stays deadlock/stall-free), while the
    attention work is emitted at high scheduler priority so the in-order PE
    queue prefers score matmuls and uses QKV matmuls as gap filler.
  - The Ln pass is batched over groups of up to 4 j-blocks via contiguous
    fp16 e-strips (amortizes the ~300ns ACT fixed cost), w² per strip in one
    DVE op, and batch-level software pipelining: scores+Exp of batch k+1 are
    emitted before the out/norm matmuls of batch k.
  - The per-chunk rsqrt epilogue is split: ACT part immediately, PE/DVE part
    (broadcast matmul, scale) deferred past the next chunk's first scores;
    the hp=1 epilogue also emits the partial projection + store for its
    rows so the projection overlaps the remaining attention.
"""

import numpy as np
import ml_dtypes

import concourse.bacc as bacc
import concourse.tile as tile
import concourse.mybir as mybir
from concourse.bass_utils import run_bass_kernel_spmd

# The act-table-set chooser assigns each activation the FIRST set containing
# its function; with the default ordering Exp -> exp_and_others and
# Ln -> natural_log, so alternating Exp/Ln thrashes ACT_TABLE_LOAD (~1.3us
# each, >100 loads). Reorder so the combined Exp+Ln set is preferred.
_orig_get_tables = bacc.get_activation_tables


def _tables_ln_exp_first(arch):
    t = _orig_get_tables(arch)
    key = "natural_log_exp_and_others"
    if key not in t:
        return t
    # Keep dict ORDER (set ids are positional); drop Exp/Ln from every other
    # set so the combined set is the unique candidate for both.
    exp = mybir.ActivationFunctionType.Exp
    ln = mybir.ActivationFunctionType.Ln
    out = {}
    for k, fns in t.items():
        out[k] = fns if k == key else (set(fns) - {exp, ln})
    return out


bacc.get_activation_tables = _tables_ln_exp_first

dt = mybir.dt
F32, F32R, F16, BF16 = dt.float32, dt.float32r, dt.float16, dt.bfloat16
AF = mybir.ActivationFunctionType

B, T, C, H, HS = 2, 2048, 1024, 16, 64
HPC = 4            # heads per core
NCORES = 8
SCALE = 1.0 / np.sqrt(HS)

_CACHE = {}


def _build():
    nc = bacc.Bacc(None, target_bir_lowering=False)

    xT = nc.dram_tensor("xT", [C, T], BF16, kind="ExternalInput")
    wqk = nc.dram_tensor("wqk", [C, 512], BF16, kind="ExternalInput")
    wv = nc.dram_tensor("wv", [C, 256], BF16, kind="ExternalInput")
    wp = nc.dram_tensor("wp", [256, C], BF16, kind="ExternalInput")
    mtri = nc.dram_tensor("mtri", [128, 128], F16, kind="ExternalInput")
    out = nc.dram_tensor("out", [T, C], BF16, kind="ExternalOutput")

    with tile.TileContext(nc) as tc:
        with (
            tc.tile_pool(name="cst", bufs=1) as cst,
            tc.tile_pool(name="data", bufs=1) as data,
            # PSUM budget (8 banks): scores 2x[128,1024] = 4, out-accum
            # 2x[128,512] = 2, shared ring "m" 2x[128,512] = 2 carrying the
            # QKV accumulators, norm accum, rsqrt broadcast and projection.
            tc.tile_pool(name="ps_s", bufs=2, space="PSUM") as ps_s,
            tc.tile_pool(name="ps_o", bufs=2, space="PSUM") as ps_o,
            tc.tile_pool(name="ps_m", bufs=2, space="PSUM") as ps_m,
            tc.tile_pool(name="we", bufs=3) as we,
            tc.tile_pool(name="epi", bufs=2) as epi,
            tc.tile_pool(name="outp", bufs=3) as outp,
        ):
            # ---- inputs: xT (4MB) + wqk gate the first scores, so they go
            # first on separate queues; wv next (needed by po ~5us later),
            # wp/mtri last.
            # host orders wqk columns [q01 | k01 | q23 | k23]; the first
            # half is the critical path to the first scores, the rest can
            # trail the xT blocks. DMA queues balanced: ~1.5MB critical each.
            wqk_sb = cst.tile([128, 8, 512], BF16)
            wqk_r = wqk[:].rearrange("(po pi) j -> pi po j", pi=128)
            nc.scalar.dma_start(wqk_sb[:, :, 0:256], wqk_r[:, :, 0:256])
            xT_sb = data.tile([128, 8, 2048], BF16)
            xT_r = xT[:].rearrange("(po pi) t -> pi po t", pi=128)
            xq = [nc.sync, nc.gpsimd, nc.scalar, nc.sync, nc.gpsimd,
                  nc.sync, nc.gpsimd, nc.scalar]
            for cb in range(8):
                xq[cb].dma_start(xT_sb[:, cb, :], xT_r[:, cb, :])
            nc.scalar.dma_start(wqk_sb[:, :, 256:512], wqk_r[:, :, 256:512])
            wv_sb = cst.tile([128, 8, 256], BF16)
            nc.gpsimd.dma_start(wv_sb, wv[:].rearrange("(po pi) j -> pi po j", pi=128))
            mtri_sb = cst.tile([128, 128], F16)
            nc.sync.dma_start(mtri_sb, mtri[:])
            wp_sb = cst.tile([128, 2, 1024], BF16)
            nc.gpsimd.dma_start(wp_sb, wp[:].rearrange("(po pi) e -> pi po e", pi=128))
            ones_n = cst.tile([128, 1], BF16)
            nc.vector.memset(ones_n, 1.0)
            ones_b = cst.tile([128, 64], BF16)
            nc.vector.memset(ones_b, 1.0)

            # ---- PE warm-up: the HAM clock gate starts at K=4/8 (1.2 GHz)
            # and needs ~3.4us of sustained busy to reach 2.4 GHz. Junk
            # matmuls during the input-DMA window warm it so the first real
            # qk matmuls (critical path to the first Exp) run at full rate.
            warm_src = data.tile([128, 512], BF16)
            nc.vector.memset(warm_src, 0.0)
            warm_ps = ps_s.tile([128, 1024], F32, tag="ps", name="warm_ps")
            for _ in range(28):
                nc.tensor.matmul(warm_ps[:, 0:512], warm_src[:, 0:128],
                                 warm_src, start=True, stop=True)

            # qkT: block 0,1 = q head-pairs; block 2,3 = k head-pairs.
            # Partition rows (h%2)*64..+64 inside each block = one head.
            qkT = data.tile([128, 4, 2048], BF16)
            v_sb = data.tile([128, 16, 256], BF16)
            yT = data.tile([128, 2, 2048], BF16)

            # ================= QKV projection feeds =================
            def emit_qk(jb, tcc):
                phys = {0: 0, 2: 1, 1: 2, 3: 3}[jb]
                pq = ps_m.tile([128, 512], F32, tag="m", name=f"pq_{jb}_{tcc}")
                for cb in range(8):
                    nc.tensor.matmul(
                        pq,
                        wqk_sb[:, cb, phys * 128:(phys + 1) * 128],
                        xT_sb[:, cb, tcc * 512:(tcc + 1) * 512],
                        start=(cb == 0), stop=(cb == 7),
                    )
                nc.vector.tensor_copy(qkT[:, jb, tcc * 512:(tcc + 1) * 512], pq)

            def emit_v(tb):
                pv = ps_m.tile([128, 256], F32, tag="m", name=f"pv_{tb}")
                for cb in range(8):
                    nc.tensor.matmul(
                        pv,
                        xT_sb[:, cb, tb * 128:(tb + 1) * 128],
                        wv_sb[:, cb, :],
                        start=(cb == 0), stop=(cb == 7),
                    )
                nc.vector.tensor_copy(v_sb[:, tb, :], pv)

            # ================= attention chunk =================
            # state: deferred epilogue/projection work, fired a couple of
            # items per batch so the PE never dumps a burst in front of the
            # next scores (which would stall the ACT queue).
            state = {"pending": []}

            def fire_pending(k=2):
                for _ in range(min(k, len(state["pending"]))):
                    state["pending"].pop(0)()

            def emit_chunk(hp, ic):
                qblk, kblk = hp, 2 + hp
                njb = 4 * ic + 4
                po = ps_o.tile([128, 512], F32, tag="po", name=f"po_{hp}_{ic}")
                pnbox = {}  # lazily allocated so the "m" ring order matches use

                def get_pn():
                    if "pn" not in pnbox:
                        pnbox["pn"] = ps_m.tile([128, 512], F32, tag="m",
                                                name=f"pn_{hp}_{ic}")
                    return pnbox["pn"]

                def emit_po_pn(prev):
                    b, w_s, w2_s, offs = prev
                    pn = get_pn()
                    for jb in b:
                        off, N = offs[jb]
                        last = jb == njb - 1
                        st = (jb == 0)
                        hA, hB = 2 * hp, 2 * hp + 1
                        nc.tensor.matmul(
                            pn[0:1, 512 - N:512], ones_n,
                            w2_s[:, off:off + N],
                            start=st, stop=last, tile_position=(0, 0),
                        )
                        nc.tensor.matmul(
                            pn[64:65, 512 - N:512], ones_n,
                            w2_s[:, off + N:off + 2 * N],
                            start=st, stop=last, tile_position=(0, 64),
                        )
                        nc.tensor.matmul(
                            po[0:64, 512 - N:512],
                            v_sb[:, jb, hA * 64:hA * 64 + 64],
                            w_s[:, off:off + N],
                            start=st, stop=last, tile_position=(0, 0),
                        )
                        nc.tensor.matmul(
                            po[64:128, 512 - N:512],
                            v_sb[:, jb, hB * 64:hB * 64 + 64],
                            w_s[:, off + N:off + 2 * N],
                            start=st, stop=last, tile_position=(0, 64),
                        )

                # batches: full j-blocks in groups of 4, then the 4
                # ragged/diagonal blocks as one batch
                batches = [list(range(k, k + 4)) for k in range(0, 4 * ic, 4)]
                batches.append(list(range(4 * ic, njb)))
                prev = None
                for b in batches:
                    e_s = we.tile([128, 4096], F16, tag="e")
                    w_s = we.tile([128, 4096], BF16, tag="w")
                    w2_s = we.tile([128, 4096], BF16, tag="w2")
                    offs = {}
                    off = 0
                    for nb, jb in enumerate(b):
                        m = jb - 4 * ic
                        N = 512 if m < 0 else 512 - 128 * m
                        ioff = ic * 512 + (512 - N)
                        ps_ = ps_s.tile([128, 1024], F32, tag="ps")
                        # scores (transposed): K=64, two heads row-packed
                        nc.tensor.matmul(
                            ps_[:, 0:N],
                            qkT[0:64, kblk, jb * 128:(jb + 1) * 128],
                            qkT[0:64, qblk, ioff:ioff + N],
                            start=True, stop=True,
                        )
                        nc.tensor.matmul(
                            ps_[:, 512:512 + N],
                            qkT[64:128, kblk, jb * 128:(jb + 1) * 128],
                            qkT[64:128, qblk, ioff:ioff + N],
                            start=True, stop=True,
                        )
                        # Exp (scale=1/8) psum -> packed fp16 strip
                        if N == 512:
                            nc.scalar.activation(
                                e_s[:, off:off + 1024], ps_, AF.Exp,
                                scale=SCALE)
                        else:
                            ps3 = ps_.rearrange(
                                "p (b n) -> p b n", b=2)[:, :, 0:N]
                            e3 = e_s[:, off:off + 2 * N].rearrange(
                                "p (b n) -> p b n", b=2)
                            nc.scalar.activation(e3, ps3, AF.Exp,
                                                 scale=SCALE)
                        offs[jb] = (off, N)
                        off += 2 * N
                        # deferred epilogue/projection work dribbles out
                        # behind the 2nd/3rd score+exp of each batch so the
                        # PE burst never blocks the next scores
                        if nb in (1, 2):
                            fire_pending(1)
                    # diag masks on the exp intermediate (before Ln):
                    # Ln(e*m + 1) = 0 on masked positions
                    for jb in b:
                        if jb - 4 * ic >= 0:
                            o, N = offs[jb]
                            nc.gpsimd.tensor_mul(
                                e_s[:, o:o + 128], e_s[:, o:o + 128],
                                mtri_sb)
                            nc.gpsimd.tensor_mul(
                                e_s[:, o + N:o + N + 128],
                                e_s[:, o + N:o + N + 128], mtri_sb)
                    # batched Ln + w^2
                    W = off
                    nc.scalar.activation(w_s[:, 0:W], e_s[:, 0:W],
                                         AF.Ln, bias=1.0)
                    for jb in b:
                        o, N = offs[jb]
                        nc.vector.tensor_mul(w2_s[:, o:o + 2 * N],
                                             w_s[:, o:o + 2 * N],
                                             w_s[:, o:o + 2 * N])
                    if prev is not None:
                        emit_po_pn(prev)
                    prev = (b, w_s, w2_s, offs)
                emit_po_pn(prev)

                # ---- chunk epilogue: y = out.T * rsqrt(norm2) ----
                # rsqrt = Exp(-0.5 * Ln(x)): stays in the Exp/Ln
                # activation-table set (no ACT_TABLE_LOAD thrash).
                # The WHOLE epilogue (including the ACT Ln/Exp of norm2) is
                # deferred into the next chunk: emitting Ln(pn) here would
                # put it before the next chunk's Exps in the in-order ACT
                # queue, stalling ACT ~2.5us per chunk on the last po/pn
                # matmuls.
                pn = get_pn()

                def run_epi(po=po, pn=pn, hp=hp, ic=ic):
                    nrm = epi.tile([128, 512], F32, tag="nrm")
                    nc.scalar.activation(nrm, pn, AF.Ln)
                    rs = epi.tile([128, 512], BF16, tag="rs")
                    nc.scalar.activation(rs, nrm, AF.Exp, scale=-0.5)
                    pb = ps_m.tile([128, 512], F32, tag="m",
                                   name=f"pb_{hp}_{ic}")
                    nc.tensor.matmul(pb[0:64, :], ones_b[0:1, :], rs[0:1, :],
                                     start=True, stop=True,
                                     tile_position=(0, 0))
                    nc.tensor.matmul(pb[64:128, :], ones_b[64:65, :],
                                     rs[64:65, :],
                                     start=True, stop=True,
                                     tile_position=(64, 64))
                    rb = epi.tile([128, 512], F32, tag="rb")
                    nc.vector.tensor_copy(rb, pb)
                    nc.vector.tensor_mul(
                        yT[:, hp, ic * 512:(ic + 1) * 512], po, rb)

                def run_proj(tcc, hp=hp):
                    # partial projection + store for one 128-row block
                    os_ = outp.tile([128, 1024], BF16, tag="os",
                                    name=f"os_{tcc}")
                    for nk in range(2):
                        pp = ps_m.tile([128, 512], F32, tag="m",
                                       name=f"pp_{tcc}_{nk}")
                        for kb in range(2):
                            nc.tensor.matmul(
                                pp,
                                yT[:, kb, tcc * 128:(tcc + 1) * 128],
                                wp_sb[:, kb, nk * 512:(nk + 1) * 512],
                                start=(kb == 0), stop=(kb == 1),
                            )
                        nc.vector.tensor_copy(
                            os_[:, nk * 512:(nk + 1) * 512], pp)
                    eng = [nc.sync, nc.scalar, nc.gpsimd][tcc % 3]
                    eng.dma_start(out[tcc * 128:(tcc + 1) * 128, :], os_)

                state["pending"].append(run_epi)
                if hp == 1:
                    for tcc in range(4 * ic, 4 * ic + 4):
                        state["pending"].append(
                            lambda tcc=tcc: run_proj(tcc))

            # ================= emission schedule =================
            # QKV feeds interleave with attention chunks: chunk (hp0, ic)
            # needs qk blocks (0,tcc<=ic), (2,tcc<=ic) and v tb<=4ic+3; the
            # attention is emitted at high priority so the PE prefers it and
            # fills its ACT-bound gaps with feed matmuls.
            emit_qk(0, 0); emit_qk(2, 0)
            for tb in range(0, 4):
                emit_v(tb)
            emit_qk(0, 1); emit_qk(2, 1)
            for tb in range(4, 8):
                emit_v(tb)
            with tc.high_priority():
                emit_chunk(0, 0)
            emit_qk(0, 2); emit_qk(2, 2)
            for tb in range(8, 12):
                emit_v(tb)
            with tc.high_priority():
                emit_chunk(0, 1)
            emit_qk(0, 3); emit_qk(2, 3)
            for tb in range(12, 16):
                emit_v(tb)
            with tc.high_priority():
                emit_chunk(0, 2)
            # hp1's first chunk only needs the t0 slices of its q/k blocks;
            # emit them before chunk (0,3) so the PE chews through them
            # during its ACT-bound stretch instead of stalling hp1's start.
            emit_qk(1, 0); emit_qk(3, 0)
            emit_qk(1, 1); emit_qk(3, 1)
            with tc.high_priority():
                emit_chunk(0, 3)
            emit_qk(1, 2); emit_qk(3, 2)
            emit_qk(1, 3); emit_qk(3, 3)
            with tc.high_priority():
                for ic in range(4):
                    emit_chunk(1, ic)
                fire_pending(len(state["pending"]))

    nc.compile()
    return nc


def _prep_inputs(x, W_qkv, W_proj):
    """Host-side shard + layout prep. Returns per-core input maps."""
    bf = ml_dtypes.bfloat16
    mtri = np.triu(np.ones((128, 128), dtype=np.float32)).astype(np.float16)
    in_maps = []
    for core in range(NCORES):
        b, g = core // 4, core % 4
        heads = range(4 * g, 4 * g + 4)
        # W_qkv rows: q = h*64.., k = C + h*64.., v = 2C + h*64..
        q_rows = np.concatenate([np.arange(h * HS, (h + 1) * HS) for h in heads])
        # column blocks ordered [q01 | k01 | q23 | k23]: the first half is
        # the critical-path DMA (first attention chunk needs heads 0,1)
        qT = W_qkv[q_rows, :].T
        kT = W_qkv[C + q_rows, :].T
        wqk = np.concatenate(
            [qT[:, 0:128], kT[:, 0:128], qT[:, 128:256], kT[:, 128:256]],
            axis=1)  # [C, 512]
        wv = W_qkv[2 * C + q_rows, :].T                            # [C, 256]
        wp = W_proj[:, q_rows].T                                   # [256, C]
        in_maps.append({
            "xT": np.ascontiguousarray(x[b].T).astype(bf),
            "wqk": np.ascontiguousarray(wqk).astype(bf),
            "wv": np.ascontiguousarray(wv).astype(bf),
            "wp": np.ascontiguousarray(wp).astype(bf),
            "mtri": mtri,
        })
    return in_maps


def _run(in_maps, trace=False, trace_cores=None):
    if "nc" not in _CACHE:
        _CACHE["nc"] = _build()
    return run_bass_kernel_spmd(
        _CACHE["nc"], in_maps, core_ids=list(range(NCORES)),
        trace=trace, trace_cores=trace_cores,
    )


def kernel(x, W_qkv, W_proj, b_proj):
    x = np.asarray(x, dtype=np.float32)
    W_qkv = np.asarray(W_qkv, dtype=np.float32)
    W_proj = np.asarray(W_proj, dtype=np.float32)
    b_proj = np.asarray(b_proj, dtype=np.float32)

    res = _run(_prep_inputs(x, W_qkv, W_proj)).results
    out = np.zeros((B, T, C), dtype=np.float64)
    for core in range(NCORES):
        out[core // 4] += np.asarray(res[core]["out"], dtype=np.float64)
    out += b_proj.astype(np.float64)
    return out.astype(np.float32)


# revision 19
# speedup vs baseline: 1.0174x; 1.0174x over previous
"""Fused sp2norm-MHA kernel for Trainium2, 8 NeuronCores.

Model (per reference):
    qkv = x @ W_qkv.T ; split heads (H=16, hs=64)
    s = (q @ k.T) / sqrt(hs);  w = softplus(s) causal-masked
    out_h = (w @ v) / ||w||_row ;  out = concat(out_h) @ W_proj.T + b_proj

Sharding: core c = (b, g) with b = c // 4 (batch), g = c % 4 (head group of 4).
Each core computes its batch's QKV for its 4 heads, the attention, and a
partial projection over its 256 feature channels. The host sums the 4 partial
projections per batch and adds the bias (the unshard step).

On-device layout (per core):
    xT  [1024, 2048]  bf16  = x[b].T                  (c on partitions)
    Sᵀ attention: scores computed transposed [j, i] (keys on partitions) so
    softplus output w feeds (a) out.T = V.T-style matmul lhsT=V[j,d], and
    (b) norm² = ones.T @ w² — both contract over j on partitions.
    softplus = Exp (scale=1/8, PSUM src) then Ln (bias=1.0), fp16 intermediate.
    Causality: block-ragged i-range per j-block; the diagonal 128-col
    sub-block is masked on the fp16 exp intermediate (Ln(e*m + 1) = 0).

v3 structure: the scalar engine (softplus Exp+Ln, ~1 elem/cycle/lane) is the
roofline at ~150us; everything is arranged so it starts early and never
stalls:
  - PSUM roles are disjoint: the score pool (4 banks) is never used by the
    QKV phase, so the first attention chunk's scores/Exp run ~10us into the
    kernel instead of waiting for the whole QKV phase to drain (the baseline
    lost ~45us to exactly this WAR-on-bank-reuse serialization).
  - Emission interleaves QKV "feeds" with attention chunks in dependency
    order (ring allocation order stays deadlock/stall-free), while the
    attention work is emitted at high scheduler priority so the in-order PE
    queue prefers score matmuls and uses QKV matmuls as gap filler.
  - The Ln pass is batched over groups of up to 4 j-blocks via contiguous
    fp16 e-strips (amortizes the ~300ns ACT fixed cost), w² per strip in one
    DVE op, and batch-level software pipelining: scores+Exp of batch k+1 are
    emitted before the out/norm matmuls of batch k.
  - The per-chunk rsqrt epilogue is split: ACT part immediately, PE/DVE part
    (broadcast matmul, scale) deferred past the next chunk's first scores;
    the hp=1 epilogue also emits the partial projection + store for its
    rows so the projection overlaps the remaining attention.
"""

import numpy as np
import ml_dtypes

import concourse.bacc as bacc
import concourse.tile as tile
import concourse.mybir as mybir
from concourse.bass_utils import run_bass_kernel_spmd

# The act-table-set chooser assigns each activation the FIRST set containing
# its function; with the default ordering Exp -> exp_and_others and
# Ln -> natural_log, so alternating Exp/Ln thrashes ACT_TABLE_LOAD (~1.3us
# each, >100 loads). Reorder so the combined Exp+Ln set is preferred.
_orig_get_tables = bacc.get_activation_tables


def _tables_ln_exp_first(arch):
    t = _orig_get_tables(arch)
    key = "natural_log_exp_and_others"
    if key not in t:
        return t
    # Keep dict ORDER (set ids are positional); drop Exp/Ln from every other
    # set so the combined set is the unique candidate for both.
    exp = mybir.ActivationFunctionType.Exp
    ln = mybir.ActivationFunctionType.Ln
    out = {}
    for k, fns in t.items():
        out[k] = fns if k == key else (set(fns) - {exp, ln})
    return out


bacc.get_activation_tables = _tables_ln_exp_first

dt = mybir.dt
F32, F32R, F16, BF16 = dt.float32, dt.float32r, dt.float16, dt.bfloat16
AF = mybir.ActivationFunctionType

B, T, C, H, HS = 2, 2048, 1024, 16, 64
HPC = 4            # heads per core
NCORES = 8
SCALE = 1.0 / np.sqrt(HS)

_CACHE = {}


def _build():
    nc = bacc.Bacc(None, target_bir_lowering=False)

    xT = nc.dram_tensor("xT", [C, T], BF16, kind="ExternalInput")
    wqk = nc.dram_tensor("wqk", [C, 512], BF16, kind="ExternalInput")
    wv = nc.dram_tensor("wv", [C, 256], BF16, kind="ExternalInput")
    wp = nc.dram_tensor("wp", [256, C], BF16, kind="ExternalInput")
    mtri = nc.dram_tensor("mtri", [128, 128], F16, kind="ExternalInput")
    out = nc.dram_tensor("out", [T, C], BF16, kind="ExternalOutput")

    with tile.TileContext(nc) as tc:
        with (
            tc.tile_pool(name="cst", bufs=1) as cst,
            tc.tile_pool(name="data", bufs=1) as data,
            # PSUM budget (8 banks): scores 2x[128,1024] = 4, out-accum
            # 2x[128,512] = 2, shared ring "m" 2x[128,512] = 2 carrying the
            # QKV accumulators, norm accum, rsqrt broadcast and projection.
            tc.tile_pool(name="ps_s", bufs=2, space="PSUM") as ps_s,
            tc.tile_pool(name="ps_o", bufs=2, space="PSUM") as ps_o,
            tc.tile_pool(name="ps_m", bufs=2, space="PSUM") as ps_m,
            tc.tile_pool(name="we", bufs=3) as we,
            tc.tile_pool(name="epi", bufs=2) as epi,
            tc.tile_pool(name="outp", bufs=3) as outp,
        ):
            # ---- inputs: xT (4MB) + wqk gate the first scores, so they go
            # first on separate queues; wv next (needed by po ~5us later),
            # wp/mtri last.
            # host orders wqk columns [q01 | k01 | q23 | k23]; the first
            # half is the critical path to the first scores, the rest can
            # trail the xT blocks. DMA queues balanced: ~1.5MB critical each.
            wqk_sb = cst.tile([128, 8, 512], BF16)
            wqk_r = wqk[:].rearrange("(po pi) j -> pi po j", pi=128)
            nc.scalar.dma_start(wqk_sb[:, :, 0:256], wqk_r[:, :, 0:256])
            xT_sb = data.tile([128, 8, 2048], BF16)
            xT_r = xT[:].rearrange("(po pi) t -> pi po t", pi=128)
            xq = [nc.sync, nc.gpsimd, nc.scalar, nc.sync, nc.gpsimd,
                  nc.sync, nc.gpsimd, nc.scalar]
            for cb in range(8):
                xq[cb].dma_start(xT_sb[:, cb, :], xT_r[:, cb, :])
            nc.scalar.dma_start(wqk_sb[:, :, 256:512], wqk_r[:, :, 256:512])
            wv_sb = cst.tile([128, 8, 256], BF16)
            nc.gpsimd.dma_start(wv_sb, wv[:].rearrange("(po pi) j -> pi po j", pi=128))
            mtri_sb = cst.tile([128, 128], F16)
            nc.sync.dma_start(mtri_sb, mtri[:])
            wp_sb = cst.tile([128, 2, 1024], BF16)
            nc.gpsimd.dma_start(wp_sb, wp[:].rearrange("(po pi) e -> pi po e", pi=128))
            ones_n = cst.tile([128, 1], BF16)
            nc.vector.memset(ones_n, 1.0)
            ones_b = cst.tile([128, 64], BF16)
            nc.vector.memset(ones_b, 1.0)

            # ---- PE warm-up: the HAM clock gate starts at K=4/8 (1.2 GHz)
            # and needs ~3.4us of sustained busy to reach 2.4 GHz. Junk
            # matmuls during the input-DMA window warm it so the first real
            # qk matmuls (critical path to the first Exp) run at full rate.
            warm_src = data.tile([128, 512], BF16)
            nc.vector.memset(warm_src, 0.0)
            warm_ps = ps_s.tile([128, 1024], F32, tag="ps", name="warm_ps")
            for _ in range(28):
                nc.tensor.matmul(warm_ps[:, 0:512], warm_src[:, 0:128],
                                 warm_src, start=True, stop=True)

            # qkT: block 0,1 = q head-pairs; block 2,3 = k head-pairs.
            # Partition rows (h%2)*64..+64 inside each block = one head.
            qkT = data.tile([128, 4, 2048], BF16)
            v_sb = data.tile([128, 16, 256], BF16)
            yT = data.tile([128, 2, 2048], BF16)

            # ================= QKV projection feeds =================
            def emit_qk(jb, tcc):
                phys = {0: 0, 2: 1, 1: 2, 3: 3}[jb]
                pq = ps_m.tile([128, 512], F32, tag="m", name=f"pq_{jb}_{tcc}")
                for cb in range(8):
                    nc.tensor.matmul(
                        pq,
                        wqk_sb[:, cb, phys * 128:(phys + 1) * 128],
                        xT_sb[:, cb, tcc * 512:(tcc + 1) * 512],
                        start=(cb == 0), stop=(cb == 7),
                    )
                nc.vector.tensor_copy(qkT[:, jb, tcc * 512:(tcc + 1) * 512], pq)

            def emit_v(tb):
                pv = ps_m.tile([128, 256], F32, tag="m", name=f"pv_{tb}")
                for cb in range(8):
                    nc.tensor.matmul(
                        pv,
                        xT_sb[:, cb, tb * 128:(tb + 1) * 128],
                        wv_sb[:, cb, :],
                        start=(cb == 0), stop=(cb == 7),
                    )
                nc.vector.tensor_copy(v_sb[:, tb, :], pv)

            # ================= attention chunk =================
            # state: deferred epilogue/projection work, fired a couple of
            # items per batch so the PE never dumps a burst in front of the
            # next scores (which would stall the ACT queue).
            state = {"pending": []}

            def fire_pending(k=2):
                for _ in range(min(k, len(state["pending"]))):
                    state["pending"].pop(0)()

            def emit_chunk(hp, ic):
                qblk, kblk = hp, 2 + hp
                njb = 4 * ic + 4
                po = ps_o.tile([128, 512], F32, tag="po", name=f"po_{hp}_{ic}")
                pnbox = {}  # lazily allocated so the "m" ring order matches use

                def get_pn():
                    if "pn" not in pnbox:
                        pnbox["pn"] = ps_m.tile([128, 512], F32, tag="m",
                                                name=f"pn_{hp}_{ic}")
                    return pnbox["pn"]

                def emit_po_pn(prev):
                    b, w_s, w2_s, offs = prev
                    pn = get_pn()
                    for jb in b:
                        off, N = offs[jb]
                        last = jb == njb - 1
                        st = (jb == 0)
                        hA, hB = 2 * hp, 2 * hp + 1
                        nc.tensor.matmul(
                            pn[0:1, 512 - N:512], ones_n,
                            w2_s[:, off:off + N],
                            start=st, stop=last, tile_position=(0, 0),
                        )
                        nc.tensor.matmul(
                            pn[64:65, 512 - N:512], ones_n,
                            w2_s[:, off + N:off + 2 * N],
                            start=st, stop=last, tile_position=(0, 64),
                        )
                        nc.tensor.matmul(
                            po[0:64, 512 - N:512],
                            v_sb[:, jb, hA * 64:hA * 64 + 64],
                            w_s[:, off:off + N],
                            start=st, stop=last, tile_position=(0, 0),
                        )
                        nc.tensor.matmul(
                            po[64:128, 512 - N:512],
                            v_sb[:, jb, hB * 64:hB * 64 + 64],
                            w_s[:, off + N:off + 2 * N],
                            start=st, stop=last, tile_position=(0, 64),
                        )

                # batches: full j-blocks in groups of 4, then the 4
                # ragged/diagonal blocks as one batch
                batches = [list(range(k, k + 4)) for k in range(0, 4 * ic, 4)]
                batches.append(list(range(4 * ic, njb)))
                prev = None
                for b in batches:
                    e_s = we.tile([128, 4096], F16, tag="e")
                    w_s = we.tile([128, 4096], BF16, tag="w")
                    w2_s = we.tile([128, 4096], BF16, tag="w2")
                    offs = {}
                    off = 0
                    for nb, jb in enumerate(b):
                        m = jb - 4 * ic
                        N = 512 if m < 0 else 512 - 128 * m
                        ioff = ic * 512 + (512 - N)
                        ps_ = ps_s.tile([128, 1024], F32, tag="ps")
                        # scores (transposed): K=64, two heads row-packed
                        nc.tensor.matmul(
                            ps_[:, 0:N],
                            qkT[0:64, kblk, jb * 128:(jb + 1) * 128],
                            qkT[0:64, qblk, ioff:ioff + N],
                            start=True, stop=True,
                        )
                        nc.tensor.matmul(
                            ps_[:, 512:512 + N],
                            qkT[64:128, kblk, jb * 128:(jb + 1) * 128],
                            qkT[64:128, qblk, ioff:ioff + N],
                            start=True, stop=True,
                        )
                        # Exp (scale=1/8) psum -> packed fp16 strip
                        if N == 512:
                            nc.scalar.activation(
                                e_s[:, off:off + 1024], ps_, AF.Exp,
                                scale=SCALE)
                        else:
                            ps3 = ps_.rearrange(
                                "p (b n) -> p b n", b=2)[:, :, 0:N]
                            e3 = e_s[:, off:off + 2 * N].rearrange(
                                "p (b n) -> p b n", b=2)
                            nc.scalar.activation(e3, ps3, AF.Exp,
                                                 scale=SCALE)
                        offs[jb] = (off, N)
                        off += 2 * N
                        # deferred epilogue/projection work dribbles out
                        # behind the 2nd/3rd score+exp of each batch so the
                        # PE burst never blocks the next scores
                        if nb in (1, 2):
                            fire_pending(1)
                    # diag masks on the exp intermediate (before Ln):
                    # Ln(e*m + 1) = 0 on masked positions
                    for jb in b:
                        if jb - 4 * ic >= 0:
                            o, N = offs[jb]
                            nc.gpsimd.tensor_mul(
                                e_s[:, o:o + 128], e_s[:, o:o + 128],
                                mtri_sb)
                            nc.gpsimd.tensor_mul(
                                e_s[:, o + N:o + N + 128],
                                e_s[:, o + N:o + N + 128], mtri_sb)
                    # batched Ln + w^2
                    W = off
                    nc.scalar.activation(w_s[:, 0:W], e_s[:, 0:W],
                                         AF.Ln, bias=1.0)
                    for jb in b:
                        o, N = offs[jb]
                        nc.vector.tensor_mul(w2_s[:, o:o + 2 * N],
                                             w_s[:, o:o + 2 * N],
                                             w_s[:, o:o + 2 * N])
                    if prev is not None:
                        emit_po_pn(prev)
                    prev = (b, w_s, w2_s, offs)
                emit_po_pn(prev)

                # ---- chunk epilogue: y = out.T * rsqrt(norm2) ----
                # rsqrt = Exp(-0.5 * Ln(x)): stays in the Exp/Ln
                # activation-table set (no ACT_TABLE_LOAD thrash).
                # The WHOLE epilogue (including the ACT Ln/Exp of norm2) is
                # deferred into the next chunk: emitting Ln(pn) here would
                # put it before the next chunk's Exps in the in-order ACT
                # queue, stalling ACT ~2.5us per chunk on the last po/pn
                # matmuls.
                pn = get_pn()

                def run_epi(po=po, pn=pn, hp=hp, ic=ic):
                    nrm = epi.tile([128, 512], F32, tag="nrm")
                    nc.scalar.activation(nrm, pn, AF.Ln)
                    rs = epi.tile([128, 512], BF16, tag="rs")
                    nc.scalar.activation(rs, nrm, AF.Exp, scale=-0.5)
                    pb = ps_m.tile([128, 512], F32, tag="m",
                                   name=f"pb_{hp}_{ic}")
                    nc.tensor.matmul(pb[0:64, :], ones_b[0:1, :], rs[0:1, :],
                                     start=True, stop=True,
                                     tile_position=(0, 0))
                    nc.tensor.matmul(pb[64:128, :], ones_b[64:65, :],
                                     rs[64:65, :],
                                     start=True, stop=True,
                                     tile_position=(64, 64))
                    rb = epi.tile([128, 512], F32, tag="rb")
                    nc.vector.tensor_copy(rb, pb)
                    nc.vector.tensor_mul(
                        yT[:, hp, ic * 512:(ic + 1) * 512], po, rb)

                def run_proj(tcc, hp=hp):
                    # partial projection + store for one 128-row block
                    os_ = outp.tile([128, 1024], BF16, tag="os",
                                    name=f"os_{tcc}")
                    for nk in range(2):
                        pp = ps_m.tile([128, 512], F32, tag="m",
                                       name=f"pp_{tcc}_{nk}")
                        for kb in range(2):
                            nc.tensor.matmul(
                                pp,
                                yT[:, kb, tcc * 128:(tcc + 1) * 128],
                                wp_sb[:, kb, nk * 512:(nk + 1) * 512],
                                start=(kb == 0), stop=(kb == 1),
                            )
                        nc.vector.tensor_copy(
                            os_[:, nk * 512:(nk + 1) * 512], pp)
                    nc.sync.dma_start(out[tcc * 128:(tcc + 1) * 128, :], os_)

                state["pending"].append(run_epi)
                if hp == 1:
                    for tcc in range(4 * ic, 4 * ic + 4):
                        state["pending"].append(
                            lambda tcc=tcc: run_proj(tcc))

            # ================= emission schedule =================
            # QKV feeds interleave with attention chunks: chunk (hp0, ic)
            # needs qk blocks (0,tcc<=ic), (2,tcc<=ic) and v tb<=4ic+3; the
            # attention is emitted at high priority so the PE prefers it and
            # fills its ACT-bound gaps with feed matmuls.
            emit_qk(0, 0); emit_qk(2, 0)
            for tb in range(0, 4):
                emit_v(tb)
            emit_qk(0, 1); emit_qk(2, 1)
            for tb in range(4, 8):
                emit_v(tb)
            with tc.high_priority():
                emit_chunk(0, 0)
            emit_qk(0, 2); emit_qk(2, 2)
            for tb in range(8, 12):
                emit_v(tb)
            with tc.high_priority():
                emit_chunk(0, 1)
            emit_qk(0, 3); emit_qk(2, 3)
            for tb in range(12, 16):
                emit_v(tb)
            with tc.high_priority():
                emit_chunk(0, 2)
            # hp1's first chunk only needs the t0 slices of its q/k blocks;
            # emit them before chunk (0,3) so the PE chews through them
            # during its ACT-bound stretch instead of stalling hp1's start.
            emit_qk(1, 0); emit_qk(3, 0)
            emit_qk(1, 1); emit_qk(3, 1)
            with tc.high_priority():
                emit_chunk(0, 3)
            emit_qk(1, 2); emit_qk(3, 2)
            emit_qk(1, 3); emit_qk(3, 3)
            with tc.high_priority():
                for ic in range(4):
                    emit_chunk(1, ic)
                fire_pending(len(state["pending"]))

    nc.compile()
    return nc


def _prep_inputs(x, W_qkv, W_proj):
    """Host-side shard + layout prep. Returns per-core input maps."""
    bf = ml_dtypes.bfloat16
    mtri = np.triu(np.ones((128, 128), dtype=np.float32)).astype(np.float16)
    in_maps = []
    for core in range(NCORES):
        b, g = core // 4, core % 4
        heads = range(4 * g, 4 * g + 4)
        # W_qkv rows: q = h*64.., k = C + h*64.., v = 2C + h*64..
        q_rows = np.concatenate([np.arange(h * HS, (h + 1) * HS) for h in heads])
        # column blocks ordered [q01 | k01 | q23 | k23]: the first half is
        # the critical-path DMA (first attention chunk needs heads 0,1)
        qT = W_qkv[q_rows, :].T
        kT = W_qkv[C + q_rows, :].T
        wqk = np.concatenate(
            [qT[:, 0:128], kT[:, 0:128], qT[:, 128:256], kT[:, 128:256]],
            axis=1)  # [C, 512]
        wv = W_qkv[2 * C + q_rows, :].T                            # [C, 256]
        wp = W_proj[:, q_rows].T                                   # [256, C]
        in_maps.append({
            "xT": np.ascontiguousarray(x[b].T).astype(bf),
            "wqk": np.ascontiguousarray(wqk).astype(bf),
            "wv": np.ascontiguousarray(wv).astype(bf),
            "wp": np.ascontiguousarray(wp).astype(bf),
            "mtri": mtri,
        })
    return in_maps


def _run(in_maps, trace=False, trace_cores=None):
    if "nc" not in _CACHE:
        _CACHE["nc"] = _build()
    return run_bass_kernel_spmd(
        _CACHE["nc"], in_maps, core_ids=list(range(NCORES)),
        trace=trace, trace_cores=trace_cores,
    )


def kernel(x, W_qkv, W_proj, b_proj):
    x = np.asarray(x, dtype=np.float32)
    W_qkv = np.asarray(W_qkv, dtype=np.float32)
    W_proj = np.asarray(W_proj, dtype=np.float32)
    b_proj = np.asarray(b_proj, dtype=np.float32)

    res = _run(_prep_inputs(x, W_qkv, W_proj)).results
    out = np.zeros((B, T, C), dtype=np.float64)
    for core in range(NCORES):
        out[core // 4] += np.asarray(res[core]["out"], dtype=np.float64)
    out += b_proj.astype(np.float64)
    return out.astype(np.float32)


# revision 20
# speedup vs baseline: 1.0518x; 1.0338x over previous
"""Fused sp2norm-MHA kernel for Trainium2, 8 NeuronCores.

Model (per reference):
    qkv = x @ W_qkv.T ; split heads (H=16, hs=64)
    s = (q @ k.T) / sqrt(hs);  w = softplus(s) causal-masked
    out_h = (w @ v) / ||w||_row ;  out = concat(out_h) @ W_proj.T + b_proj

Sharding: core c = (b, g) with b = c // 4 (batch), g = c % 4 (head group of 4).
Each core computes its batch's QKV for its 4 heads, the attention, and a
partial projection over its 256 feature channels. The host sums the 4 partial
projections per batch and adds the bias (the unshard step).

On-device layout (per core):
    xT  [1024, 2048]  bf16  = x[b].T                  (c on partitions)
    Sᵀ attention: scores computed transposed [j, i] (keys on partitions) so
    softplus output w feeds (a) out.T = V.T-style matmul lhsT=V[j,d], and
    (b) norm² = ones.T @ w² — both contract over j on partitions.
    softplus = Exp (scale=1/8, PSUM src) then Ln (bias=1.0), fp16 intermediate.
    Causality: block-ragged i-range per j-block; the diagonal 128-col
    sub-block is masked on the fp16 exp intermediate (Ln(e*m + 1) = 0).

v3 structure: the scalar engine (softplus Exp+Ln, ~1 elem/cycle/lane) is the
roofline at ~150us; everything is arranged so it starts early and never
stalls:
  - PSUM roles are disjoint: the score pool (4 banks) is never used by the
    QKV phase, so the first attention chunk's scores/Exp run ~10us into the
    kernel instead of waiting for the whole QKV phase to drain (the baseline
    lost ~45us to exactly this WAR-on-bank-reuse serialization).
  - Emission interleaves QKV "feeds" with attention chunks in dependency
    order (ring allocation order stays deadlock/stall-free), while the
    attention work is emitted at high scheduler priority so the in-order PE
    queue prefers score matmuls and uses QKV matmuls as gap filler.
  - The Ln pass is batched over groups of up to 4 j-blocks via contiguous
    fp16 e-strips (amortizes the ~300ns ACT fixed cost), w² per strip in one
    DVE op, and batch-level software pipelining: scores+Exp of batch k+1 are
    emitted before the out/norm matmuls of batch k.
  - The per-chunk rsqrt epilogue is split: ACT part immediately, PE/DVE part
    (broadcast matmul, scale) deferred past the next chunk's first scores;
    the hp=1 epilogue also emits the partial projection + store for its
    rows so the projection overlaps the remaining attention.
"""

import numpy as np
import ml_dtypes

import concourse.bacc as bacc
import concourse.tile as tile
import concourse.mybir as mybir
from concourse.bass_utils import run_bass_kernel_spmd

# The act-table-set chooser assigns each activation the FIRST set containing
# its function; with the default ordering Exp -> exp_and_others and
# Ln -> natural_log, so alternating Exp/Ln thrashes ACT_TABLE_LOAD (~1.3us
# each, >100 loads). Reorder so the combined Exp+Ln set is preferred.
_orig_get_tables = bacc.get_activation_tables


def _tables_ln_exp_first(arch):
    t = _orig_get_tables(arch)
    key = "natural_log_exp_and_others"
    if key not in t:
        return t
    # Keep dict ORDER (set ids are positional); drop Exp/Ln from every other
    # set so the combined set is the unique candidate for both.
    exp = mybir.ActivationFunctionType.Exp
    ln = mybir.ActivationFunctionType.Ln
    out = {}
    for k, fns in t.items():
        out[k] = fns if k == key else (set(fns) - {exp, ln})
    return out


bacc.get_activation_tables = _tables_ln_exp_first

dt = mybir.dt
F32, F32R, F16, BF16 = dt.float32, dt.float32r, dt.float16, dt.bfloat16
AF = mybir.ActivationFunctionType

B, T, C, H, HS = 2, 2048, 1024, 16, 64
HPC = 4            # heads per core
NCORES = 8
SCALE = 1.0 / np.sqrt(HS)

_CACHE = {}


def _build():
    nc = bacc.Bacc(None, target_bir_lowering=False)

    xT = nc.dram_tensor("xT", [C, T], BF16, kind="ExternalInput")
    wqk = nc.dram_tensor("wqk", [C, 512], BF16, kind="ExternalInput")
    wv = nc.dram_tensor("wv", [C, 256], BF16, kind="ExternalInput")
    wp = nc.dram_tensor("wp", [256, C], BF16, kind="ExternalInput")
    mtri = nc.dram_tensor("mtri", [128, 128], F16, kind="ExternalInput")
    out = nc.dram_tensor("out", [T, C], BF16, kind="ExternalOutput")

    with tile.TileContext(nc) as tc:
        with (
            tc.tile_pool(name="cst", bufs=1) as cst,
            tc.tile_pool(name="data", bufs=1) as data,
            # PSUM budget (8 banks): scores 2x[128,1024] = 4, out-accum
            # 2x[128,512] = 2, shared ring "m" 2x[128,512] = 2 carrying the
            # QKV accumulators, norm accum, rsqrt broadcast and projection.
            tc.tile_pool(name="ps_s", bufs=2, space="PSUM") as ps_s,
            tc.tile_pool(name="ps_o", bufs=2, space="PSUM") as ps_o,
            tc.tile_pool(name="ps_m", bufs=2, space="PSUM") as ps_m,
            tc.tile_pool(name="we", bufs=3) as we,
            tc.tile_pool(name="epi", bufs=2) as epi,
            tc.tile_pool(name="outp", bufs=3) as outp,
        ):
            # ---- inputs: xT (4MB) + wqk gate the first scores, so they go
            # first on separate queues; wv next (needed by po ~5us later),
            # wp/mtri last.
            # host orders wqk columns [q01 | k01 | q23 | k23]; the first
            # half is the critical path to the first scores, the rest can
            # trail the xT blocks. DMA queues balanced: ~1.5MB critical each.
            wqk_sb = cst.tile([128, 8, 512], BF16)
            wqk_r = wqk[:].rearrange("(po pi) j -> pi po j", pi=128)
            nc.scalar.dma_start(wqk_sb[:, :, 0:256], wqk_r[:, :, 0:256])
            xT_sb = data.tile([128, 8, 2048], BF16)
            xT_r = xT[:].rearrange("(po pi) t -> pi po t", pi=128)
            xq = [nc.sync, nc.gpsimd, nc.scalar, nc.sync, nc.gpsimd,
                  nc.sync, nc.gpsimd, nc.scalar]
            for cb in range(8):
                xq[cb].dma_start(xT_sb[:, cb, :], xT_r[:, cb, :])
            nc.scalar.dma_start(wqk_sb[:, :, 256:512], wqk_r[:, :, 256:512])
            wv_sb = cst.tile([128, 8, 256], BF16)
            nc.gpsimd.dma_start(wv_sb, wv[:].rearrange("(po pi) j -> pi po j", pi=128))
            mtri_sb = cst.tile([128, 128], F16)
            nc.sync.dma_start(mtri_sb, mtri[:])
            wp_sb = cst.tile([128, 2, 1024], BF16)
            nc.gpsimd.dma_start(wp_sb, wp[:].rearrange("(po pi) e -> pi po e", pi=128))
            ones_n = cst.tile([128, 1], BF16)
            nc.vector.memset(ones_n, 1.0)
            ones_b = cst.tile([128, 64], BF16)
            nc.vector.memset(ones_b, 1.0)

            # ---- PE warm-up: the HAM clock gate starts at K=4/8 (1.2 GHz)
            # and needs ~3.4us of sustained busy to reach 2.4 GHz. Junk
            # matmuls during the input-DMA window warm it so the first real
            # qk matmuls (critical path to the first Exp) run at full rate.
            warm_src = data.tile([128, 512], BF16)
            nc.vector.memset(warm_src, 0.0)
            warm_ps = ps_s.tile([128, 1024], F32, tag="ps", name="warm_ps")
            for _ in range(28):
                nc.tensor.matmul(warm_ps[:, 0:512], warm_src[:, 0:128],
                                 warm_src, start=True, stop=True)

            # qkT: block 0,1 = q head-pairs; block 2,3 = k head-pairs.
            # Partition rows (h%2)*64..+64 inside each block = one head.
            qkT = data.tile([128, 4, 2048], BF16)
            v_sb = data.tile([128, 16, 256], BF16)
            yT = data.tile([128, 2, 2048], BF16)

            # ================= QKV projection feeds =================
            def emit_qk(jb, tcc):
                phys = {0: 0, 2: 1, 1: 2, 3: 3}[jb]
                pq = ps_m.tile([128, 512], F32, tag="m", name=f"pq_{jb}_{tcc}")
                for cb in range(8):
                    nc.tensor.matmul(
                        pq,
                        wqk_sb[:, cb, phys * 128:(phys + 1) * 128],
                        xT_sb[:, cb, tcc * 512:(tcc + 1) * 512],
                        start=(cb == 0), stop=(cb == 7),
                    )
                nc.vector.tensor_copy(qkT[:, jb, tcc * 512:(tcc + 1) * 512], pq)

            def emit_v(tb):
                pv = ps_m.tile([128, 256], F32, tag="m", name=f"pv_{tb}")
                for cb in range(8):
                    nc.tensor.matmul(
                        pv,
                        xT_sb[:, cb, tb * 128:(tb + 1) * 128],
                        wv_sb[:, cb, :],
                        start=(cb == 0), stop=(cb == 7),
                    )
                nc.vector.tensor_copy(v_sb[:, tb, :], pv)

            # ================= attention chunk =================
            # state: deferred epilogue/projection work, fired a couple of
            # items per batch so the PE never dumps a burst in front of the
            # next scores (which would stall the ACT queue).
            state = {"pending": []}

            def fire_pending(k=2):
                for _ in range(min(k, len(state["pending"]))):
                    state["pending"].pop(0)()

            def emit_chunk(hp, ic):
                qblk, kblk = hp, 2 + hp
                njb = 4 * ic + 4
                po = ps_o.tile([128, 512], F32, tag="po", name=f"po_{hp}_{ic}")
                pnbox = {}  # lazily allocated so the "m" ring order matches use

                def get_pn():
                    if "pn" not in pnbox:
                        pnbox["pn"] = ps_m.tile([128, 512], F32, tag="m",
                                                name=f"pn_{hp}_{ic}")
                    return pnbox["pn"]

                def emit_po_pn(prev):
                    b, w_s, w2_s, offs = prev
                    pn = get_pn()
                    for jb in b:
                        off, N = offs[jb]
                        last = jb == njb - 1
                        st = (jb == 0)
                        hA, hB = 2 * hp, 2 * hp + 1
                        nc.tensor.matmul(
                            pn[0:1, 512 - N:512], ones_n,
                            w2_s[:, off:off + N],
                            start=st, stop=last, tile_position=(0, 0),
                        )
                        nc.tensor.matmul(
                            pn[64:65, 512 - N:512], ones_n,
                            w2_s[:, off + N:off + 2 * N],
                            start=st, stop=last, tile_position=(0, 64),
                        )
                        nc.tensor.matmul(
                            po[0:64, 512 - N:512],
                            v_sb[:, jb, hA * 64:hA * 64 + 64],
                            w_s[:, off:off + N],
                            start=st, stop=last, tile_position=(0, 0),
                        )
                        nc.tensor.matmul(
                            po[64:128, 512 - N:512],
                            v_sb[:, jb, hB * 64:hB * 64 + 64],
                            w_s[:, off + N:off + 2 * N],
                            start=st, stop=last, tile_position=(0, 64),
                        )

                # batches: full j-blocks in groups of 4, then the 4
                # ragged/diagonal blocks as one batch
                batches = [list(range(k, k + 4)) for k in range(0, 4 * ic, 4)]
                batches.append(list(range(4 * ic, njb)))
                prev = None
                for b in batches:
                    e_s = we.tile([128, 4096], F16, tag="e")
                    w_s = we.tile([128, 4096], BF16, tag="w")
                    w2_s = we.tile([128, 4096], BF16, tag="w2")
                    offs = {}
                    off = 0
                    for nb, jb in enumerate(b):
                        m = jb - 4 * ic
                        N = 512 if m < 0 else 512 - 128 * m
                        ioff = ic * 512 + (512 - N)
                        ps_ = ps_s.tile([128, 1024], F32, tag="ps")
                        # scores (transposed): K=64, two heads row-packed
                        nc.tensor.matmul(
                            ps_[:, 0:N],
                            qkT[0:64, kblk, jb * 128:(jb + 1) * 128],
                            qkT[0:64, qblk, ioff:ioff + N],
                            start=True, stop=True,
                        )
                        nc.tensor.matmul(
                            ps_[:, 512:512 + N],
                            qkT[64:128, kblk, jb * 128:(jb + 1) * 128],
                            qkT[64:128, qblk, ioff:ioff + N],
                            start=True, stop=True,
                        )
                        # Exp (scale=1/8) psum -> packed fp16 strip
                        if N == 512:
                            nc.scalar.activation(
                                e_s[:, off:off + 1024], ps_, AF.Exp,
                                scale=SCALE)
                        else:
                            ps3 = ps_.rearrange(
                                "p (b n) -> p b n", b=2)[:, :, 0:N]
                            e3 = e_s[:, off:off + 2 * N].rearrange(
                                "p (b n) -> p b n", b=2)
                            nc.scalar.activation(e3, ps3, AF.Exp,
                                                 scale=SCALE)
                        # diag mask on the exp intermediate, right after
                        # its own Exp so the DVE FIFO wait is progressive:
                        # Ln(e*m + 1) = 0 on masked positions
                        if m >= 0:
                            nc.vector.tensor_mul(
                                e_s[:, off:off + 128], e_s[:, off:off + 128],
                                mtri_sb)
                            nc.vector.tensor_mul(
                                e_s[:, off + N:off + N + 128],
                                e_s[:, off + N:off + N + 128], mtri_sb)
                        offs[jb] = (off, N)
                        off += 2 * N
                        # deferred epilogue/projection work dribbles out
                        # behind the 3rd/4th score+exp of each batch so the
                        # PE burst never blocks the next scores and the
                        # epilogue Ln never heads the ACT queue too early
                        if nb in (2, 3):
                            fire_pending(1)
                    # batched Ln + w^2
                    W = off
                    nc.scalar.activation(w_s[:, 0:W], e_s[:, 0:W],
                                         AF.Ln, bias=1.0)
                    for jb in b:
                        o, N = offs[jb]
                        nc.vector.tensor_mul(w2_s[:, o:o + 2 * N],
                                             w_s[:, o:o + 2 * N],
                                             w_s[:, o:o + 2 * N])
                    if prev is not None:
                        emit_po_pn(prev)
                    prev = (b, w_s, w2_s, offs)
                emit_po_pn(prev)

                # ---- chunk epilogue: y = out.T * rsqrt(norm2) ----
                # rsqrt = Exp(-0.5 * Ln(x)): stays in the Exp/Ln
                # activation-table set (no ACT_TABLE_LOAD thrash).
                # The WHOLE epilogue (including the ACT Ln/Exp of norm2) is
                # deferred into the next chunk: emitting Ln(pn) here would
                # put it before the next chunk's Exps in the in-order ACT
                # queue, stalling ACT ~2.5us per chunk on the last po/pn
                # matmuls.
                pn = get_pn()

                def run_epi(po=po, pn=pn, hp=hp, ic=ic):
                    nrm = epi.tile([128, 512], F32, tag="nrm")
                    nc.scalar.activation(nrm, pn, AF.Ln)
                    rs = epi.tile([128, 512], BF16, tag="rs")
                    nc.scalar.activation(rs, nrm, AF.Exp, scale=-0.5)
                    pb = ps_m.tile([128, 512], F32, tag="m",
                                   name=f"pb_{hp}_{ic}")
                    nc.tensor.matmul(pb[0:64, :], ones_b[0:1, :], rs[0:1, :],
                                     start=True, stop=True,
                                     tile_position=(0, 0))
                    nc.tensor.matmul(pb[64:128, :], ones_b[64:65, :],
                                     rs[64:65, :],
                                     start=True, stop=True,
                                     tile_position=(64, 64))
                    rb = epi.tile([128, 512], F32, tag="rb")
                    nc.vector.tensor_copy(rb, pb)
                    nc.vector.tensor_mul(
                        yT[:, hp, ic * 512:(ic + 1) * 512], po, rb)

                def run_proj(tcc, hp=hp):
                    # partial projection + store for one 128-row block
                    os_ = outp.tile([128, 1024], BF16, tag="os",
                                    name=f"os_{tcc}")
                    for nk in range(2):
                        pp = ps_m.tile([128, 512], F32, tag="m",
                                       name=f"pp_{tcc}_{nk}")
                        for kb in range(2):
                            nc.tensor.matmul(
                                pp,
                                yT[:, kb, tcc * 128:(tcc + 1) * 128],
                                wp_sb[:, kb, nk * 512:(nk + 1) * 512],
                                start=(kb == 0), stop=(kb == 1),
                            )
                        nc.vector.tensor_copy(
                            os_[:, nk * 512:(nk + 1) * 512], pp)
                    nc.sync.dma_start(out[tcc * 128:(tcc + 1) * 128, :], os_)

                state["pending"].append(run_epi)
                if hp == 1:
                    for tcc in range(4 * ic, 4 * ic + 4):
                        state["pending"].append(
                            lambda tcc=tcc: run_proj(tcc))

            # ================= emission schedule =================
            # QKV feeds interleave with attention chunks: chunk (hp0, ic)
            # needs qk blocks (0,tcc<=ic), (2,tcc<=ic) and v tb<=4ic+3; the
            # attention is emitted at high priority so the PE prefers it and
            # fills its ACT-bound gaps with feed matmuls.
            emit_qk(0, 0); emit_qk(2, 0)
            for tb in range(0, 4):
                emit_v(tb)
            emit_qk(0, 1); emit_qk(2, 1)
            for tb in range(4, 8):
                emit_v(tb)
            with tc.high_priority():
                emit_chunk(0, 0)
            emit_qk(0, 2); emit_qk(2, 2)
            for tb in range(8, 12):
                emit_v(tb)
            with tc.high_priority():
                emit_chunk(0, 1)
            emit_qk(0, 3); emit_qk(2, 3)
            for tb in range(12, 16):
                emit_v(tb)
            with tc.high_priority():
                emit_chunk(0, 2)
            # hp1's first chunk only needs the t0 slices of its q/k blocks;
            # emit them before chunk (0,3) so the PE chews through them
            # during its ACT-bound stretch instead of stalling hp1's start.
            emit_qk(1, 0); emit_qk(3, 0)
            emit_qk(1, 1); emit_qk(3, 1)
            with tc.high_priority():
                emit_chunk(0, 3)
            emit_qk(1, 2); emit_qk(3, 2)
            emit_qk(1, 3); emit_qk(3, 3)
            with tc.high_priority():
                for ic in range(4):
                    emit_chunk(1, ic)
                fire_pending(len(state["pending"]))

    nc.compile()
    return nc


def _prep_inputs(x, W_qkv, W_proj):
    """Host-side shard + layout prep. Returns per-core input maps."""
    bf = ml_dtypes.bfloat16
    mtri = np.triu(np.ones((128, 128), dtype=np.float32)).astype(np.float16)
    in_maps = []
    for core in range(NCORES):
        b, g = core // 4, core % 4
        heads = range(4 * g, 4 * g + 4)
        # W_qkv rows: q = h*64.., k = C + h*64.., v = 2C + h*64..
        q_rows = np.concatenate([np.arange(h * HS, (h + 1) * HS) for h in heads])
        # column blocks ordered [q01 | k01 | q23 | k23]: the first half is
        # the critical-path DMA (first attention chunk needs heads 0,1)
        qT = W_qkv[q_rows, :].T
        kT = W_qkv[C + q_rows, :].T
        wqk = np.concatenate(
            [qT[:, 0:128], kT[:, 0:128], qT[:, 128:256], kT[:, 128:256]],
            axis=1)  # [C, 512]
        wv = W_qkv[2 * C + q_rows, :].T                            # [C, 256]
        wp = W_proj[:, q_rows].T                                   # [256, C]
        in_maps.append({
            "xT": np.ascontiguousarray(x[b].T).astype(bf),
            "wqk": np.ascontiguousarray(wqk).astype(bf),
            "wv": np.ascontiguousarray(wv).astype(bf),
            "wp": np.ascontiguousarray(wp).astype(bf),
            "mtri": mtri,
        })
    return in_maps


def _run(in_maps, trace=False, trace_cores=None):
    if "nc" not in _CACHE:
        _CACHE["nc"] = _build()
    return run_bass_kernel_spmd(
        _CACHE["nc"], in_maps, core_ids=list(range(NCORES)),
        trace=trace, trace_cores=trace_cores,
    )


def kernel(x, W_qkv, W_proj, b_proj):
    x = np.asarray(x, dtype=np.float32)
    W_qkv = np.asarray(W_qkv, dtype=np.float32)
    W_proj = np.asarray(W_proj, dtype=np.float32)
    b_proj = np.asarray(b_proj, dtype=np.float32)

    res = _run(_prep_inputs(x, W_qkv, W_proj)).results
    out = np.zeros((B, T, C), dtype=np.float64)
    for core in range(NCORES):
        out[core // 4] += np.asarray(res[core]["out"], dtype=np.float64)
    out += b_proj.astype(np.float64)
    return out.astype(np.float32)
